# revision 28
# baseline (speedup 1.0000x reference)
"""PlainGCN message passing on 8 TRN2 NeuronCores.

Computation (reference):
    deg = bincount(h); dis = deg**-0.5
    out[t] = relu(sum_{e: t_e=t} dis[t]*dis[h_e] * x[h_e])
           = relu(dis[t] * sum_e dis[h_e] * x[h_e])     (separable norm)

Device strategy (per core, SPMD shared program):
  - Dest nodes are assigned to (core, tile, pos) slots by a host-side
    balancer so each (core, tile, bucket) edge cell is <= 192
    (= E/(8 cores * 98 tiles * 4 buckets) rounded up to 64, the PE
    base-partition quantum). Edge runs then need almost no padding and
    all eight cores share one static schedule.
  - x is split into 4 source buckets at edge-count quantiles (~25k nodes
    each, so int16 gather indices fit). Per (tile-group, bucket) span,
    rows are fetched with gpsimd.dma_gather; bucket b uses SWDGE queue b
    so descriptor generation runs on all four Q7 core pairs concurrently
    (queue q is served by Q7 cores 2q/2q+1).
  - ScalarE casts gathered rows fp32->bf16; VectorE builds per-column
    one-hot(dest-pos)*dis[h] bf16 matrices; TensorE segment-reduces via
    bf16 matmul accumulation in PSUM; ScalarE applies relu with the
    per-dest dis[t] as the activation scale; DMA out.
"""

import ml_dtypes
import numpy as np

import concourse.bacc as bacc
import concourse.mybir as mybir
import concourse.tile as tile
from concourse.bass_utils import run_bass_kernel_spmd
from concourse.library_config import mlp as mlp_lib

P = 128
N_CORES = 8
N_BUCKETS = 4
TILES_PER_GROUP = 8


def _quantile_buckets(h, n, e):
    """Split node ids into N_BUCKETS contiguous ranges with ~equal edge
    counts (and < 32768 nodes each, so gather idx fits int16)."""
    cnt = np.bincount(h, minlength=n)
    cum = np.cumsum(cnt)
    cuts = [
        int(np.searchsorted(cum, (e * (i + 1)) // N_BUCKETS, side="left")) + 1
        for i in range(N_BUCKETS - 1)
    ]
    bstart = np.array([0] + cuts, dtype=np.int64)
    bend = np.array(cuts + [n], dtype=np.int64)
    rows = bend - bstart
    assert (rows > 0).all() and (rows <= 32768).all(), rows
    return bstart, bend


def _balance_assign(deg_tb, n_tiles, cap_e):
    """Assign each node to one of n_cores*n_tiles cells (<=128 nodes each)
    keeping per-(cell,bucket) edge sums <= cap_e where possible.

    Best-fit-decreasing: for each node (heaviest bucket-degree first) pick
    the feasible cell minimizing the resulting max bucket load, with a
    node-count pressure term so node slots also fill evenly. When no cell
    is feasible, pick the cell minimizing the projected 64-quantum padding
    increase (clusters overflow into already-padded cells).

    Returns (cell_of[n], pos_of[n], counts[ncells, nb], n_violations).
    """
    n, nb = deg_tb.shape
    ncells = N_CORES * n_tiles
    order = np.argsort(-deg_tb.max(axis=1), kind="stable")
    loads = np.zeros((ncells, nb), np.int32)
    nnode = np.zeros(ncells, np.int32)
    cell_of = np.empty(n, np.int32)
    pos_of = np.empty(n, np.int32)
    j_cell = np.arange(ncells, dtype=np.int64) % n_tiles
    # per-(tile, bucket) 64-quantum run ceiling = max over cores
    cur_ceil = np.zeros((n_tiles, nb), np.int32)
    BIG = np.float32(1e9)
    viol = 0
    soft = cap_e - 3  # reserve headroom for the placement tail
    for t_node in order:
        dv = deg_tb[t_node].astype(np.int32)
        new = loads + dv
        has_slot = nnode < P
        feasible = has_slot & (new <= soft).all(axis=1)
        if not feasible.any():
            feasible = has_slot & (new <= cap_e).all(axis=1)
        if feasible.any():
            score = new.max(axis=1).astype(np.float32) + nnode * np.float32(0.5)
            score[~feasible] = BIG
            c = int(np.argmin(score))
        else:
            # choose the cell minimizing the e_pad increase: runs are
            # padded to 64*max over cores, so overflow prefers (tile,
            # bucket) runs that are already bumped
            viol += 1
            new_ceil = -(-new // 64)
            delta = np.maximum(new_ceil - cur_ceil[j_cell], 0).sum(axis=1)
            score = delta.astype(np.float32) * np.float32(1e4) \
                + new.max(axis=1)
            score[~has_slot] = BIG
            c = int(np.argmin(score))
        cell_of[t_node] = c
        pos_of[t_node] = nnode[c]
        nnode[c] += 1
        loads[c] += dv
        jj = c % n_tiles
        np.maximum(cur_ceil[jj], -(-loads[c] // 64), out=cur_ceil[jj])

    # repair pass: move light nodes out of over-cap cells into cells with
    # room, so runs drop back to the cap quantum
    nodes_in_cell = [[] for _ in range(ncells)]
    for t_node in order:
        nodes_in_cell[cell_of[t_node]].append(t_node)
    stuck: set[int] = set()
    for _ in range(8000):
        over_mask = (loads > cap_e).any(axis=1)
        for c in stuck:
            over_mask[c] = False
        over = np.nonzero(over_mask)[0]
        if len(over) == 0:
            break
        c = int(over[0])
        bover = int(np.argmax(loads[c]))
        # shed the lightest node that has an edge in the over bucket
        cand = [tn for tn in nodes_in_cell[c] if deg_tb[tn, bover] > 0]
        cand.sort(key=lambda tn: int(deg_tb[tn].sum()))
        moved = False
        for tn in cand:
            dv = deg_tb[tn].astype(np.int32)
            new = loads + dv
            ok = (nnode < P) & (new <= cap_e).all(axis=1)
            ok[c] = False
            if ok.any():
                score = new.max(axis=1).astype(np.float32)
                score[~ok] = BIG
                d_cell = int(np.argmin(score))
                nodes_in_cell[c].remove(tn)
                nodes_in_cell[d_cell].append(tn)
                loads[c] -= dv
                loads[d_cell] += dv
                nnode[c] -= 1
                nnode[d_cell] += 1
                cell_of[tn] = d_cell
                moved = True
                break
        if not moved:
            stuck.add(c)

    # recompute positions after repair
    nnode[:] = 0
    for t_node in order:
        c = cell_of[t_node]
        pos_of[t_node] = nnode[c]
        nnode[c] += 1

    counts = loads.reshape(N_CORES, n_tiles, nb)
    return cell_of, pos_of, counts, viol


def _preprocess(x, h, t):
    n, d = x.shape
    e = h.shape[0]
    h = np.asarray(h).astype(np.int64)
    t = np.asarray(t).astype(np.int64)

    # one spare tile per core gives the balancer ~1.4% node-slot slack so
    # per-(cell,bucket) caps can be met without overflow
    n_tiles = -(-n // (N_CORES * P)) + 1  # 99 dest tiles per core
    npc_slots = n_tiles * P  # 12672 dest slots per core

    deg = np.bincount(h, minlength=n).astype(np.float32)
    dis = np.where(deg > 0, deg, 1).astype(np.float32) ** np.float32(-0.5)

    bstart, bend = _quantile_buckets(h, n, e)
    b = np.searchsorted(bend, h, side="right").astype(np.int64)  # edge bucket
    gidx_all = (h - bstart[b]).astype(np.int16)

    # balanced dest-node -> (core, tile, pos) assignment
    deg_tb = np.zeros((n, N_BUCKETS), np.int64)
    np.add.at(deg_tb, (t, b), 1)
    cap_e = 64 * (-(-e // (N_CORES * n_tiles * N_BUCKETS * 64)))  # 192
    cell_of, pos_of, counts, viol = _balance_assign(deg_tb, n_tiles, cap_e)

    core_of_node = (cell_of // n_tiles).astype(np.int64)
    tile_of_node = (cell_of % n_tiles).astype(np.int64)

    core = core_of_node[t]
    j = tile_of_node[t]
    tin = pos_of[t].astype(np.float32)
    dish = dis[h]

    run_len = 64 * (-(-counts.max(axis=0) // 64))  # [n_tiles, nb]

    n_groups = -(-n_tiles // TILES_PER_GROUP)
    groups = [
        list(range(g * TILES_PER_GROUP, min((g + 1) * TILES_PER_GROUP, n_tiles)))
        for g in range(n_groups)
    ]

    # span (g, b) covers runs (j in groups[g], b), padded to a multiple of P
    spans = []  # (g, b, start, length) in stream coords
    seg_lists = [[] for _ in range(n_tiles)]  # per tile: (col, p0, k, b)
    run_start = np.zeros((n_tiles, N_BUCKETS), dtype=np.int64)
    pos = 0
    for g, tiles_g in enumerate(groups):
        for bb in range(N_BUCKETS):
            s0 = pos
            for jj in tiles_g:
                run_start[jj, bb] = pos
                r = int(run_len[jj, bb])
                q = pos
                while q < pos + r:
                    k = min(P - (q % P), pos + r - q)
                    assert q % P in (0, 64) and k in (64, P), (q, k)
                    seg_lists[jj].append((q // P, q % P, k, bb))
                    q += k
                pos += r
            pos = -(-pos // P) * P
            spans.append((g, bb, s0, pos - s0))
    e_pad = pos
    n_cols = e_pad // P

    # per-core data arrays in stream order
    order_key = (core * n_groups + (j // TILES_PER_GROUP)) * N_BUCKETS * n_tiles \
        + b * n_tiles + j
    sort_idx = np.argsort(order_key, kind="stable")
    per_core = []
    for c in range(N_CORES):
        sel = sort_idx[core[sort_idx] == c]
        jj = j[sel]
        bb2 = b[sel]
        key = jj * N_BUCKETS + bb2
        # sel is sorted by (g, b, j) which refines (j, b) groups contiguously
        change = np.r_[True, (key[1:] != key[:-1])] if len(sel) else np.array([], bool)
        grp_id = np.cumsum(change) - 1 if len(sel) else change
        first_pos = np.nonzero(change)[0]
        within = np.arange(len(sel)) - first_pos[grp_id] if len(sel) else change
        posn = run_start[jj, bb2] + within

        gi = np.zeros(e_pad, dtype=np.int16)
        tf = np.zeros(e_pad, dtype=np.float32)
        nf = np.zeros(e_pad, dtype=np.float32)
        gi[posn] = gidx_all[sel]
        tf[posn] = tin[sel]
        nf[posn] = dish[sel]

        # wrap gather indices: [16, len/16] per span, tiled x8 partitions
        wrap = np.zeros((P, e_pad // 16), dtype=np.int16)
        for (_g, _b, s0, ln) in spans:
            if ln == 0:
                continue
            w0 = s0 // 16
            seg = gi[s0:s0 + ln].reshape(ln // 16, 16).T
            wrap[:, w0:w0 + ln // 16] = np.tile(seg, (8, 1))

        tlocF = tf.reshape(n_cols, P).T
        dishF = nf.reshape(n_cols, P).T
        meta = np.concatenate([tlocF, dishF], axis=1)  # [128, 2C] f32

        # per-(tile, pos) dis[t] scale for the fused relu; 0 on empty slots
        dist = np.zeros((P, n_tiles), dtype=np.float32)
        mine = np.nonzero(core_of_node == c)[0]
        dist[pos_of[mine], tile_of_node[mine]] = dis[mine]

        per_core.append({"gidx": wrap, "meta": meta, "dist": dist})

    iota = np.tile(np.arange(P, dtype=np.float32), (P, 1)).astype(
        ml_dtypes.bfloat16)

    # output gather map: full[node] = y_concat[core*npc_slots + tile*128+pos]
    out_index = core_of_node * npc_slots + tile_of_node * P + pos_of

    schedule = {
        "n": n, "d": d, "npc_slots": npc_slots, "n_tiles": n_tiles,
        "n_cols": n_cols, "e_pad": e_pad, "bstart": bstart, "bend": bend,
        "groups": groups, "spans": spans, "seg_lists": seg_lists,
        "viol": viol,
    }
    return schedule, per_core, iota, out_index


def _build_program(sched, n_cores):
    n, d = sched["n"], sched["d"]
    npc_slots, n_tiles = sched["npc_slots"], sched["n_tiles"]
    n_cols, e_pad = sched["n_cols"], sched["e_pad"]
    bstart, bend = sched["bstart"], sched["bend"]
    groups, spans, seg_lists = sched["groups"], sched["spans"], sched["seg_lists"]

    nc = bacc.Bacc("TRN2", target_bir_lowering=False, debug=False,
                   num_devices=n_cores, num_swdge_queues=4)
    f32 = mybir.dt.float32
    bf16 = mybir.dt.bfloat16
    x_d = nc.dram_tensor("x", [n, d], f32, kind="ExternalInput")
    iota_d = nc.dram_tensor("iota", [P, P], bf16, kind="ExternalInput")
    gidx_d = nc.dram_tensor("gidx", [P, e_pad // 16], mybir.dt.int16,
                            kind="ExternalInput")
    meta_d = nc.dram_tensor("meta", [P, 2 * n_cols], f32, kind="ExternalInput")
    dist_d = nc.dram_tensor("dist", [P, n_tiles], f32, kind="ExternalInput")
    y_d = nc.dram_tensor("y", [npc_slots, d], f32, kind="ExternalOutput")

    nc.gpsimd.load_library(mlp_lib)

    max_span = max(ln for (_g, _b, _s, ln) in spans)
    span_by_gb = {(g, b): (s0, ln) for (g, b, s0, ln) in spans}
    relu = mybir.ActivationFunctionType.Relu
    ident = mybir.ActivationFunctionType.Identity

    with tile.TileContext(nc) as tc:
        with (
            tc.tile_pool(name="const", bufs=1) as cpool,
            tc.tile_pool(name="gather", bufs=12) as gpool,
            tc.tile_pool(name="gatherb", bufs=8) as gbpool,
            tc.tile_pool(name="onehot", bufs=10) as opool,
            tc.tile_pool(name="psum", bufs=8, space="PSUM") as ppool,
            tc.tile_pool(name="outs", bufs=6) as ypool,
        ):
            # load gidx in two chunks so group 0/1 gathers start early
            head_end = min(spans[min(2 * N_BUCKETS, len(spans)) - 1][2]
                           + spans[min(2 * N_BUCKETS, len(spans)) - 1][3],
                           e_pad)
            gidx_t = cpool.tile([P, e_pad // 16], mybir.dt.int16, tag="gidx")
            gidx_head = cpool.tile([P, head_end // 16], mybir.dt.int16,
                                   tag="gidxh")
            nc.sync.dma_start(gidx_head[:], gidx_d[:, :head_end // 16])
            iota_t = cpool.tile([P, P], bf16, tag="iota")
            nc.sync.dma_start(iota_t[:], iota_d[:, :])
            meta_t = cpool.tile([P, 2 * n_cols], f32, tag="meta")
            nc.sync.dma_start(meta_t[:], meta_d[:, :])
            nc.sync.dma_start(gidx_t[:, head_end // 16:],
                              gidx_d[:, head_end // 16:])
            dist_t = cpool.tile([P, n_tiles], f32, tag="dist")
            nc.sync.dma_start(dist_t[:], dist_d[:, :])

            gtiles = {}  # (g, b) -> (bf16 tile, s0)
            ohs = {}  # (g, b) -> (bf16 one-hot span tile, c0)

            def prep_group(g):
                """Gathers (Pool), scale-casts (DVE), one-hots (DVE) for
                group g - all independent of other groups' processing."""
                for b in range(N_BUCKETS):
                    s0, ln = span_by_gb[(g, b)]
                    if ln == 0:
                        continue
                    C = ln // P
                    c0 = s0 // P
                    base = int(bstart[b])
                    rows = int(bend[b]) - base
                    gt = gpool.tile([P, (max_span // P) * d], f32, tag="gt",
                                    name=f"gt{g}_{b}")
                    gt_3d = gt[:, :C * d].rearrange("p (c d) -> p c d", d=d)
                    gsrc = gidx_head if s0 + ln <= head_end else gidx_t
                    nc.gpsimd.dma_gather(
                        gt_3d,
                        x_d[base:base + rows, :],
                        gsrc[:, s0 // 16:(s0 + ln) // 16],
                        ln, ln, d,
                        single_packet=False,
                        queue_num=b,
                    )
                    # scaled one-hot: oh[p, c*128+f] =
                    #   (iota[f] == tloc[p, c]) * dish[p, c]
                    # (dish=0 on pad slots kills their rows). One fused
                    # tensor_scalar per column halves DVE SBUF-port traffic
                    # vs two span-wide passes - the ports are shared with
                    # the Q7 SWDGE descriptor rings, so extra DVE traffic
                    # slows gather descriptor generation directly.
                    oh = opool.tile([P, max_span], bf16, tag="oh",
                                    name=f"oh{g}_{b}")
                    for cl in range(C):
                        nc.vector.tensor_scalar(
                            oh[:, cl * P:(cl + 1) * P], iota_t[:],
                            meta_t[:, c0 + cl:c0 + cl + 1],
                            meta_t[:, n_cols + c0 + cl:n_cols + c0 + cl + 1],
                            mybir.AluOpType.is_equal,
                            mybir.AluOpType.mult,
                        )
                    # plain cast fp32->bf16 on ScalarE
                    gtb = gbpool.tile([P, (max_span // P) * d], bf16,
                                      tag="gtb", name=f"gtb{g}_{b}")
                    nc.scalar.activation(gtb[:, :C * d], gt[:, :C * d], ident)
                    gtiles[(g, b)] = (gtb, s0)
                    ohs[(g, b)] = (oh, c0)

            def process_group(g):
                tiles_g = groups[g]
                ystage = None
                ystart = None
                nstage = 0
                for ji, jj in enumerate(tiles_g):
                    if ystage is None:
                        nstage = min(4, len(tiles_g) - ji)
                        ystage = ypool.tile([P, nstage * d], f32, tag="yt",
                                            name=f"yt{jj}")
                        ystart = jj
                    segs = seg_lists[jj]
                    # one PSUM tile per dest tile: the base-0 chain is one
                    # accumulation group (start=True); the base-64 chain
                    # accumulates onto the same tile as a second group
                    # (start=False). Base partition stays constant WITHIN
                    # each group, which is the PE constraint.
                    chains = [s for s in
                              ([x for x in segs if x[1] == 0],
                               [x for x in segs if x[1] == 64]) if s]
                    yt = ystage[:, (jj - ystart) * d:(jj - ystart + 1) * d]
                    scale = dist_t[:, jj:jj + 1]
                    if not chains:
                        nc.vector.memset(yt, 0.0)
                    else:
                        pt = ppool.tile([P, d], f32, tag="ps",
                                        name=f"ps{jj}")
                        for ci, ss in enumerate(chains):
                            for si, (col, p0, k, b) in enumerate(ss):
                                oh, oh_c0 = ohs[(g, b)]
                                gtb, s0 = gtiles[(g, b)]
                                col_l = col - s0 // P
                                nc.tensor.matmul(
                                    pt[:],
                                    lhsT=oh[p0:p0 + k,
                                            col_l * P:(col_l + 1) * P],
                                    rhs=gtb[p0:p0 + k,
                                            col_l * d:(col_l + 1) * d],
                                    start=(ci == 0 and si == 0),
                                    stop=(ci == len(chains) - 1
                                          and si == len(ss) - 1),
                                    skip_group_check=len(chains) > 1,
                                )
                        nc.scalar.activation(yt, pt[:], relu, scale=scale)
                    if jj - ystart + 1 == nstage:
                        nc.sync.dma_start(
                            y_d[ystart * P:(ystart + nstage) * P, :].rearrange(
                                "(i p) f -> p i f", p=P),
                            ystage[:].rearrange("p (i f) -> p i f", f=d),
                        )
                        ystage = None

            n_groups = len(groups)
            prep_group(0)
            if n_groups > 1:
                prep_group(1)
            for g in range(n_groups):
                if g + 2 < n_groups:
                    prep_group(g + 2)
                process_group(g)

    nc.compile()
    return nc


def _run(x, h, t, n_cores=N_CORES, trace=False):
    import time
    t0 = time.monotonic()
    x = np.ascontiguousarray(np.asarray(x, dtype=np.float32))
    sched, per_core, iota, out_index = _preprocess(x, h, t)
    t1 = time.monotonic()
    print(f"[kernel] preprocess {t1 - t0:.1f}s  e_pad={sched['e_pad']} "
          f"cols={sched['n_cols']} viol={sched['viol']}", flush=True)
    nc = _build_program(sched, n_cores)
    t2 = time.monotonic()
    print(f"[kernel] build+tile-schedule {t2 - t1:.1f}s", flush=True)
    in_maps = [
        {"x": x, "iota": iota, "gidx": pc["gidx"], "meta": pc["meta"],
         "dist": pc["dist"]}
        for pc in per_core
    ]
    res = run_bass_kernel_spmd(nc, in_maps, core_ids=list(range(n_cores)),
                               trace=trace)
    t3 = time.monotonic()
    print(f"[kernel] compile+run {t3 - t2:.1f}s", flush=True)
    y_all = np.concatenate([res.results[c]["y"] for c in range(n_cores)], axis=0)
    y = y_all[out_index]
    return y, res


def kernel(x, h, t):
    y, _ = _run(np.asarray(x), np.asarray(h), np.asarray(t))
    return y


# revision 29
# speedup vs baseline: 1.2437x; 1.2437x over previous
"""PlainGCN message passing on 8 TRN2 NeuronCores.

Computation (reference):
    deg = bincount(h); dis = deg**-0.5
    out[t] = relu(sum_{e: t_e=t} dis[t]*dis[h_e] * x[h_e])
           = relu(dis[t] * sum_e dis[h_e] * x[h_e])     (separable norm)

Device strategy (per core, SPMD shared program):
  - Dest nodes are assigned to (core, tile, pos) slots by a host-side
    balancer so each (core, tile, bucket) edge cell is <= 192
    (= E/(8 cores * 98 tiles * 4 buckets) rounded up to 64, the PE
    base-partition quantum). Edge runs then need almost no padding and
    all eight cores share one static schedule.
  - x is split into 4 source buckets at edge-count quantiles (~25k nodes
    each, so int16 gather indices fit). Per (tile-group, bucket) span,
    rows are fetched with gpsimd.dma_gather; bucket b uses SWDGE queue b
    so descriptor generation runs on all four Q7 core pairs concurrently
    (queue q is served by Q7 cores 2q/2q+1).
  - ScalarE casts gathered rows fp32->bf16; VectorE builds per-column
    one-hot(dest-pos)*dis[h] bf16 matrices; TensorE segment-reduces via
    bf16 matmul accumulation in PSUM; ScalarE applies relu with the
    per-dest dis[t] as the activation scale; DMA out.
"""

import ml_dtypes
import numpy as np

import concourse.bacc as bacc
import concourse.mybir as mybir
import concourse.tile as tile
from concourse.bass_utils import run_bass_kernel_spmd
from concourse.library_config import mlp as mlp_lib

P = 128
N_CORES = 8
N_BUCKETS = 4
TILES_PER_GROUP = 8


def _quantile_buckets(h, n, e):
    """Split node ids into N_BUCKETS contiguous ranges with ~equal edge
    counts (and < 32768 nodes each, so gather idx fits int16)."""
    cnt = np.bincount(h, minlength=n)
    cum = np.cumsum(cnt)
    cuts = [
        int(np.searchsorted(cum, (e * (i + 1)) // N_BUCKETS, side="left")) + 1
        for i in range(N_BUCKETS - 1)
    ]
    bstart = np.array([0] + cuts, dtype=np.int64)
    bend = np.array(cuts + [n], dtype=np.int64)
    rows = bend - bstart
    assert (rows > 0).all() and (rows <= 32768).all(), rows
    return bstart, bend


def _balance_assign(deg_tb, n_tiles, cap_e):
    """Assign each node to one of n_cores*n_tiles cells (<=128 nodes each)
    keeping per-(cell,bucket) edge sums <= cap_e where possible.

    Best-fit-decreasing: for each node (heaviest bucket-degree first) pick
    the feasible cell minimizing the resulting max bucket load, with a
    node-count pressure term so node slots also fill evenly. When no cell
    is feasible, pick the cell minimizing the projected 64-quantum padding
    increase (clusters overflow into already-padded cells).

    Returns (cell_of[n], pos_of[n], counts[ncells, nb], n_violations).
    """
    n, nb = deg_tb.shape
    ncells = N_CORES * n_tiles
    order = np.argsort(-deg_tb.max(axis=1), kind="stable")
    loads = np.zeros((ncells, nb), np.int32)
    nnode = np.zeros(ncells, np.int32)
    cell_of = np.empty(n, np.int32)
    pos_of = np.empty(n, np.int32)
    j_cell = np.arange(ncells, dtype=np.int64) % n_tiles
    # per-(tile, bucket) 64-quantum run ceiling = max over cores
    cur_ceil = np.zeros((n_tiles, nb), np.int32)
    BIG = np.float32(1e9)
    viol = 0
    soft = cap_e - 3  # reserve headroom for the placement tail
    for t_node in order:
        dv = deg_tb[t_node].astype(np.int32)
        new = loads + dv
        has_slot = nnode < P
        feasible = has_slot & (new <= soft).all(axis=1)
        if not feasible.any():
            feasible = has_slot & (new <= cap_e).all(axis=1)
        if feasible.any():
            score = new.max(axis=1).astype(np.float32) + nnode * np.float32(0.5)
            score[~feasible] = BIG
            c = int(np.argmin(score))
        else:
            # choose the cell minimizing the e_pad increase: runs are
            # padded to 64*max over cores, so overflow prefers (tile,
            # bucket) runs that are already bumped
            viol += 1
            new_ceil = -(-new // 64)
            delta = np.maximum(new_ceil - cur_ceil[j_cell], 0).sum(axis=1)
            score = delta.astype(np.float32) * np.float32(1e4) \
                + new.max(axis=1)
            score[~has_slot] = BIG
            c = int(np.argmin(score))
        cell_of[t_node] = c
        pos_of[t_node] = nnode[c]
        nnode[c] += 1
        loads[c] += dv
        jj = c % n_tiles
        np.maximum(cur_ceil[jj], -(-loads[c] // 64), out=cur_ceil[jj])

    # repair pass: move light nodes out of over-cap cells into cells with
    # room, so runs drop back to the cap quantum
    nodes_in_cell = [[] for _ in range(ncells)]
    for t_node in order:
        nodes_in_cell[cell_of[t_node]].append(t_node)
    stuck: set[int] = set()
    for _ in range(8000):
        over_mask = (loads > cap_e).any(axis=1)
        for c in stuck:
            over_mask[c] = False
        over = np.nonzero(over_mask)[0]
        if len(over) == 0:
            break
        c = int(over[0])
        bover = int(np.argmax(loads[c]))
        # shed the lightest node that has an edge in the over bucket
        cand = [tn for tn in nodes_in_cell[c] if deg_tb[tn, bover] > 0]
        cand.sort(key=lambda tn: int(deg_tb[tn].sum()))
        moved = False
        for tn in cand:
            dv = deg_tb[tn].astype(np.int32)
            new = loads + dv
            ok = (nnode < P) & (new <= cap_e).all(axis=1)
            ok[c] = False
            if ok.any():
                score = new.max(axis=1).astype(np.float32)
                score[~ok] = BIG
                d_cell = int(np.argmin(score))
                nodes_in_cell[c].remove(tn)
                nodes_in_cell[d_cell].append(tn)
                loads[c] -= dv
                loads[d_cell] += dv
                nnode[c] -= 1
                nnode[d_cell] += 1
                cell_of[tn] = d_cell
                moved = True
                break
        if not moved:
            stuck.add(c)

    # recompute positions after repair
    nnode[:] = 0
    for t_node in order:
        c = cell_of[t_node]
        pos_of[t_node] = nnode[c]
        nnode[c] += 1

    counts = loads.reshape(N_CORES, n_tiles, nb)
    return cell_of, pos_of, counts, viol


def _preprocess(x, h, t):
    n, d = x.shape
    e = h.shape[0]
    h = np.asarray(h).astype(np.int64)
    t = np.asarray(t).astype(np.int64)

    # one spare tile per core gives the balancer ~1.4% node-slot slack so
    # per-(cell,bucket) caps can be met without overflow
    n_tiles = -(-n // (N_CORES * P)) + 1  # 99 dest tiles per core
    npc_slots = n_tiles * P  # 12672 dest slots per core

    deg = np.bincount(h, minlength=n).astype(np.float32)
    dis = np.where(deg > 0, deg, 1).astype(np.float32) ** np.float32(-0.5)

    bstart, bend = _quantile_buckets(h, n, e)
    b = np.searchsorted(bend, h, side="right").astype(np.int64)  # edge bucket
    gidx_all = (h - bstart[b]).astype(np.int16)

    # balanced dest-node -> (core, tile, pos) assignment
    deg_tb = np.zeros((n, N_BUCKETS), np.int64)
    np.add.at(deg_tb, (t, b), 1)
    cap_e = 64 * (-(-e // (N_CORES * n_tiles * N_BUCKETS * 64)))  # 192
    cell_of, pos_of, counts, viol = _balance_assign(deg_tb, n_tiles, cap_e)

    core_of_node = (cell_of // n_tiles).astype(np.int64)
    tile_of_node = (cell_of % n_tiles).astype(np.int64)

    core = core_of_node[t]
    j = tile_of_node[t]
    tin = pos_of[t].astype(np.float32)
    dish = dis[h]

    run_len = 64 * (-(-counts.max(axis=0) // 64))  # [n_tiles, nb]

    n_groups = -(-n_tiles // TILES_PER_GROUP)
    groups = [
        list(range(g * TILES_PER_GROUP, min((g + 1) * TILES_PER_GROUP, n_tiles)))
        for g in range(n_groups)
    ]

    # span (g, b) covers runs (j in groups[g], b), padded to a multiple of P
    spans = []  # (g, b, start, length) in stream coords
    seg_lists = [[] for _ in range(n_tiles)]  # per tile: (col, p0, k, b)
    run_start = np.zeros((n_tiles, N_BUCKETS), dtype=np.int64)
    pos = 0
    for g, tiles_g in enumerate(groups):
        for bb in range(N_BUCKETS):
            s0 = pos
            for jj in tiles_g:
                run_start[jj, bb] = pos
                r = int(run_len[jj, bb])
                q = pos
                while q < pos + r:
                    k = min(P - (q % P), pos + r - q)
                    assert q % P in (0, 64) and k in (64, P), (q, k)
                    seg_lists[jj].append((q // P, q % P, k, bb))
                    q += k
                pos += r
            pos = -(-pos // P) * P
            spans.append((g, bb, s0, pos - s0))
    e_pad = pos
    n_cols = e_pad // P

    # per-core data arrays in stream order
    order_key = (core * n_groups + (j // TILES_PER_GROUP)) * N_BUCKETS * n_tiles \
        + b * n_tiles + j
    sort_idx = np.argsort(order_key, kind="stable")
    per_core = []
    for c in range(N_CORES):
        sel = sort_idx[core[sort_idx] == c]
        jj = j[sel]
        bb2 = b[sel]
        key = jj * N_BUCKETS + bb2
        # sel is sorted by (g, b, j) which refines (j, b) groups contiguously
        change = np.r_[True, (key[1:] != key[:-1])] if len(sel) else np.array([], bool)
        grp_id = np.cumsum(change) - 1 if len(sel) else change
        first_pos = np.nonzero(change)[0]
        within = np.arange(len(sel)) - first_pos[grp_id] if len(sel) else change
        posn = run_start[jj, bb2] + within

        gi = np.zeros(e_pad, dtype=np.int16)
        tf = np.zeros(e_pad, dtype=np.float32)
        nf = np.zeros(e_pad, dtype=np.float32)
        gi[posn] = gidx_all[sel]
        tf[posn] = tin[sel]
        nf[posn] = dish[sel]

        # wrap gather indices: [16, len/16] per span, tiled x8 partitions
        wrap = np.zeros((P, e_pad // 16), dtype=np.int16)
        for (_g, _b, s0, ln) in spans:
            if ln == 0:
                continue
            w0 = s0 // 16
            seg = gi[s0:s0 + ln].reshape(ln // 16, 16).T
            wrap[:, w0:w0 + ln // 16] = np.tile(seg, (8, 1))

        tlocF = tf.reshape(n_cols, P).T
        dishF = nf.reshape(n_cols, P).T
        meta = np.concatenate([tlocF, dishF], axis=1).astype(
            ml_dtypes.bfloat16)  # [128, 2C] bf16

        # per-(tile, pos) dis[t] scale for the fused relu; 0 on empty slots
        dist = np.zeros((P, n_tiles), dtype=np.float32)
        mine = np.nonzero(core_of_node == c)[0]
        dist[pos_of[mine], tile_of_node[mine]] = dis[mine]

        per_core.append({"gidx": wrap, "meta": meta, "dist": dist})

    iota = np.tile(np.arange(P, dtype=np.float32), (P, 1)).astype(
        ml_dtypes.bfloat16)

    # output gather map: full[node] = y_concat[core*npc_slots + tile*128+pos]
    out_index = core_of_node * npc_slots + tile_of_node * P + pos_of

    schedule = {
        "n": n, "d": d, "npc_slots": npc_slots, "n_tiles": n_tiles,
        "n_cols": n_cols, "e_pad": e_pad, "bstart": bstart, "bend": bend,
        "groups": groups, "spans": spans, "seg_lists": seg_lists,
        "viol": viol,
    }
    return schedule, per_core, iota, out_index


def _build_program(sched, n_cores):
    n, d = sched["n"], sched["d"]
    npc_slots, n_tiles = sched["npc_slots"], sched["n_tiles"]
    n_cols, e_pad = sched["n_cols"], sched["e_pad"]
    bstart, bend = sched["bstart"], sched["bend"]
    groups, spans, seg_lists = sched["groups"], sched["spans"], sched["seg_lists"]

    nc = bacc.Bacc("TRN2", target_bir_lowering=False, debug=False,
                   num_devices=n_cores, num_swdge_queues=4)
    f32 = mybir.dt.float32
    bf16 = mybir.dt.bfloat16
    x_d = nc.dram_tensor("x", [n, d], f32, kind="ExternalInput")
    iota_d = nc.dram_tensor("iota", [P, P], bf16, kind="ExternalInput")
    gidx_d = nc.dram_tensor("gidx", [P, e_pad // 16], mybir.dt.int16,
                            kind="ExternalInput")
    meta_d = nc.dram_tensor("meta", [P, 2 * n_cols], bf16, kind="ExternalInput")
    dist_d = nc.dram_tensor("dist", [P, n_tiles], f32, kind="ExternalInput")
    y_d = nc.dram_tensor("y", [npc_slots, d], f32, kind="ExternalOutput")

    nc.gpsimd.load_library(mlp_lib)

    max_span = max(ln for (_g, _b, _s, ln) in spans)
    span_by_gb = {(g, b): (s0, ln) for (g, b, s0, ln) in spans}
    relu = mybir.ActivationFunctionType.Relu
    ident = mybir.ActivationFunctionType.Identity

    with tile.TileContext(nc) as tc:
        with (
            tc.tile_pool(name="const", bufs=1) as cpool,
            tc.tile_pool(name="gather", bufs=12) as gpool,
            tc.tile_pool(name="gatherb", bufs=10) as gbpool,
            tc.tile_pool(name="onehot", bufs=10) as opool,
            tc.tile_pool(name="psum", bufs=8, space="PSUM") as ppool,
            tc.tile_pool(name="outs", bufs=6) as ypool,
        ):
            # load gidx in two chunks so group 0/1 gathers start early
            head_end = min(spans[min(2 * N_BUCKETS, len(spans)) - 1][2]
                           + spans[min(2 * N_BUCKETS, len(spans)) - 1][3],
                           e_pad)
            gidx_t = cpool.tile([P, e_pad // 16], mybir.dt.int16, tag="gidx")
            gidx_head = cpool.tile([P, head_end // 16], mybir.dt.int16,
                                   tag="gidxh")
            nc.sync.dma_start(gidx_head[:], gidx_d[:, :head_end // 16])
            iota_t = cpool.tile([P, P], bf16, tag="iota")
            nc.sync.dma_start(iota_t[:], iota_d[:, :])
            meta_t = cpool.tile([P, 2 * n_cols], bf16, tag="meta")
            nc.sync.dma_start(meta_t[:], meta_d[:, :])
            nc.sync.dma_start(gidx_t[:, head_end // 16:],
                              gidx_d[:, head_end // 16:])
            dist_t = cpool.tile([P, n_tiles], f32, tag="dist")
            nc.sync.dma_start(dist_t[:], dist_d[:, :])

            gtiles = {}  # (g, b) -> (bf16 tile, s0)
            ohs = {}  # (g, b) -> (bf16 one-hot span tile, c0)

            def prep_group(g):
                """Gathers (Pool), scale-casts (DVE), one-hots (DVE) for
                group g - all independent of other groups' processing."""
                for b in range(N_BUCKETS):
                    s0, ln = span_by_gb[(g, b)]
                    if ln == 0:
                        continue
                    C = ln // P
                    c0 = s0 // P
                    base = int(bstart[b])
                    rows = int(bend[b]) - base
                    gt = gpool.tile([P, (max_span // P) * d], f32, tag="gt",
                                    name=f"gt{g}_{b}")
                    gt_3d = gt[:, :C * d].rearrange("p (c d) -> p c d", d=d)
                    gsrc = gidx_head if s0 + ln <= head_end else gidx_t
                    nc.gpsimd.dma_gather(
                        gt_3d,
                        x_d[base:base + rows, :],
                        gsrc[:, s0 // 16:(s0 + ln) // 16],
                        ln, ln, d,
                        single_packet=False,
                        queue_num=b,
                    )
                    # scaled one-hot: oh[p, c*128+f] =
                    #   (iota[f] == tloc[p, c]) * dish[p, c]
                    # (dish=0 on pad slots kills their rows)
                    oh = opool.tile([P, max_span], bf16, tag="oh",
                                    name=f"oh{g}_{b}")
                    oh3 = oh[:, :ln].rearrange("p (c f) -> p c f", f=P)
                    nc.vector.tensor_tensor(
                        oh3, iota_t[:, None, :].broadcast_to([P, C, P]),
                        meta_t[:, c0:c0 + C, None].broadcast_to([P, C, P]),
                        mybir.AluOpType.is_equal)
                    nc.vector.tensor_tensor(
                        oh3, oh3,
                        meta_t[:, n_cols + c0:n_cols + c0 + C,
                               None].broadcast_to([P, C, P]),
                        mybir.AluOpType.mult)
                    # plain cast fp32->bf16 on ScalarE
                    gtb = gbpool.tile([P, (max_span // P) * d], bf16,
                                      tag="gtb", name=f"gtb{g}_{b}")
                    nc.scalar.activation(gtb[:, :C * d], gt[:, :C * d], ident)
                    gtiles[(g, b)] = (gtb, s0)
                    ohs[(g, b)] = (oh, c0)

            def process_group(g):
                tiles_g = groups[g]
                ystage = None
                ystart = None
                nstage = 0
                for ji, jj in enumerate(tiles_g):
                    if ystage is None:
                        nstage = min(4, len(tiles_g) - ji)
                        ystage = ypool.tile([P, nstage * d], f32, tag="yt",
                                            name=f"yt{jj}")
                        ystart = jj
                    segs = seg_lists[jj]
                    ps = {}
                    for base in (0, 64):
                        ss = [s for s in segs if s[1] == base]
                        if not ss:
                            continue
                        pt = ppool.tile([P, d], f32, tag="ps",
                                        name=f"ps{jj}_{base}")
                        ps[base] = pt
                        for si, (col, p0, k, b) in enumerate(ss):
                            oh, oh_c0 = ohs[(g, b)]
                            gtb, s0 = gtiles[(g, b)]
                            col_l = col - s0 // P
                            nc.tensor.matmul(
                                pt[:],
                                lhsT=oh[p0:p0 + k,
                                        col_l * P:(col_l + 1) * P],
                                rhs=gtb[p0:p0 + k, col_l * d:(col_l + 1) * d],
                                start=(si == 0),
                                stop=(si == len(ss) - 1),
                            )
                    yt = ystage[:, (jj - ystart) * d:(jj - ystart + 1) * d]
                    scale = dist_t[:, jj:jj + 1]
                    if 0 in ps and 64 in ps:
                        s64 = ypool.tile([P, d], f32, tag="s64",
                                         name=f"s64_{jj}")
                        nc.scalar.activation(s64[:], ps[64][:], ident)
                        st = ypool.tile([P, d], f32, tag="st", name=f"st{jj}")
                        nc.vector.tensor_add(st[:], s64[:], ps[0][:])
                        nc.scalar.activation(yt, st[:], relu, scale=scale)
                    elif 0 in ps:
                        nc.scalar.activation(yt, ps[0][:], relu, scale=scale)
                    elif 64 in ps:
                        nc.scalar.activation(yt, ps[64][:], relu, scale=scale)
                    else:
                        nc.vector.memset(yt, 0.0)
                    if jj - ystart + 1 == nstage:
                        nc.sync.dma_start(
                            y_d[ystart * P:(ystart + nstage) * P, :].rearrange(
                                "(i p) f -> p i f", p=P),
                            ystage[:].rearrange("p (i f) -> p i f", f=d),
                        )
                        ystage = None

            n_groups = len(groups)
            prep_group(0)
            if n_groups > 1:
                prep_group(1)
            for g in range(n_groups):
                if g + 2 < n_groups:
                    prep_group(g + 2)
                process_group(g)

    nc.compile()
    return nc


def _run(x, h, t, n_cores=N_CORES, trace=False):
    import time
    t0 = time.monotonic()
    x = np.ascontiguousarray(np.asarray(x, dtype=np.float32))
    sched, per_core, iota, out_index = _preprocess(x, h, t)
    t1 = time.monotonic()
    print(f"[kernel] preprocess {t1 - t0:.1f}s  e_pad={sched['e_pad']} "
          f"cols={sched['n_cols']} viol={sched['viol']}", flush=True)
    nc = _build_program(sched, n_cores)
    t2 = time.monotonic()
    print(f"[kernel] build+tile-schedule {t2 - t1:.1f}s", flush=True)
    in_maps = [
        {"x": x, "iota": iota, "gidx": pc["gidx"], "meta": pc["meta"],
         "dist": pc["dist"]}
        for pc in per_core
    ]
    res = run_bass_kernel_spmd(nc, in_maps, core_ids=list(range(n_cores)),
                               trace=trace)
    t3 = time.monotonic()
    print(f"[kernel] compile+run {t3 - t2:.1f}s", flush=True)
    y_all = np.concatenate([res.results[c]["y"] for c in range(n_cores)], axis=0)
    y = y_all[out_index]
    return y, res


def kernel(x, h, t):
    y, _ = _run(np.asarray(x), np.asarray(h), np.asarray(t))
    return y
